# revision 9
# baseline (speedup 1.0000x reference)
"""Trainium2 Bass kernel for a 12-layer BERT encoder with per-sample annotator head.

Strategy: pure data parallelism over the batch (32 samples -> 4 per NeuronCore,
8 cores, zero collectives). Activations live in SBUF in transposed [H, T]
layout for the whole 12-layer stack so no transposes are ever needed:
  - X @ W computed as matmul(lhsT=W, rhs=X^T) -> (XW)^T
  - V computed naturally ([t, d]) via matmul(lhsT=X^T, rhs=Wv)
  - scores^T ([k, q]) via matmul(lhsT=K^T, rhs=Q^T); softmax over the
    partition (k) axis with no max-subtraction (scores are small by
    construction), denominator via a ones-column appended to V.
Embedding gather + embedding LayerNorm and the tiny [50,768,2] annotator head
run on host (memory-bound pre/post-processing; the compute-regime transformer
stack runs on device in bf16 with fp32 PSUM accumulation).
"""

import os
import sys
from contextlib import ExitStack

for _p in ("/opt/trn_rl_repo", "/root/.axon_site/_ro/trn_rl_repo"):
    if os.path.isdir(_p) and _p not in sys.path:
        sys.path.insert(0, _p)

import numpy as np
import ml_dtypes

import concourse.bass as bass
import concourse.mybir as mybir
import concourse.tile as tile
from concourse import bacc
from concourse.bass import ds
from concourse.bass_utils import run_bass_kernel_spmd

DT16 = mybir.dt.float16
F32 = mybir.dt.float32
AF = mybir.ActivationFunctionType
ALU = mybir.AluOpType

L, H, NH, DH, DFF = 12, 768, 12, 64, 3072
B, S = 32, 512
NCORES = 8
B_LOC = B // NCORES            # samples per core
T = B_LOC * S                  # tokens per core
HT = H // 128                  # 6 h-tiles
FT = DFF // 128                # 24 f-tiles
CHUNK = 2 * S                  # attention/LN chunk (2 samples)
NCHUNK = T // CHUNK
EPS = 1e-12

# vec[:, col] layout (per-partition f32 vectors, one [12,128,78] dram tensor)
C_BQ, C_BK, C_BV, C_BO = 0, 6, 12, 18
C_B1, C_B2 = 24, 48
C_L1G, C_L1B, C_L2G, C_L2B = 54, 60, 66, 72
NVEC = 78


def build_nc(n_layers=L):
    nc = bacc.Bacc("TRN2", target_bir_lowering=False, debug=False)

    x0_d = nc.dram_tensor("x0", [H, T], DT16, kind="ExternalInput")
    wqkvo_d = nc.dram_tensor("wqkvo", [n_layers * 4 * H, H], DT16, kind="ExternalInput")
    w1_d = nc.dram_tensor("w1", [n_layers * H, DFF], DT16, kind="ExternalInput")
    w2_d = nc.dram_tensor("w2", [n_layers * DFF, H], DT16, kind="ExternalInput")
    vec_d = nc.dram_tensor("vec", [n_layers * 128, NVEC], F32, kind="ExternalInput")
    bvrow_d = nc.dram_tensor("bvrow", [n_layers, H], DT16, kind="ExternalInput")
    maskb_d = nc.dram_tensor("maskb", [128, B_LOC * 4], F32, kind="ExternalInput")
    out_d = nc.dram_tensor("out", [HT, 128, B_LOC], F32, kind="ExternalOutput")

    with tile.TileContext(nc) as tc, ExitStack() as ctx:
        enter = ctx.enter_context
        persist = enter(tc.tile_pool(name="persist", bufs=1))
        wmat = enter(tc.tile_pool(name="wmat", bufs=2))
        w1p = enter(tc.tile_pool(name="w1p", bufs=2))
        w2p = enter(tc.tile_pool(name="w2p", bufs=2))
        vecp = enter(tc.tile_pool(name="vecp", bufs=2))
        qkpool = enter(tc.tile_pool(name="qk", bufs=1))
        vep = enter(tc.tile_pool(name="vep", bufs=1))
        ctxp = enter(tc.tile_pool(name="ctxp", bufs=1))
        residp = enter(tc.tile_pool(name="resid", bufs=1))
        x1p = enter(tc.tile_pool(name="x1p", bufs=1))
        htp = enter(tc.tile_pool(name="htp", bufs=1))
        epool = enter(tc.tile_pool(name="ep", bufs=1))
        small = enter(tc.tile_pool(name="small", bufs=2))
        bcastp = enter(tc.tile_pool(name="bcastp", bufs=2))
        pp = enter(tc.tile_pool(name="pp", bufs=5, space="PSUM"))
        pstat = enter(tc.tile_pool(name="pstat", bufs=1, space="PSUM"))

        # ---------------- persistent state ----------------
        x = persist.tile([128, HT, T], DT16)          # activations, transposed
        ones1 = persist.tile([1, 128], DT16)          # K=1 lhsT for V bias
        ones128 = persist.tile([128, 1], DT16)        # lhsT for LN stat sums
        epst = persist.tile([1, 1], F32)
        maskb = persist.tile([128, B_LOC * 4], F32)

        nc.sync.dma_start(x[:], x0_d.rearrange("(c p) t -> p c t", p=128))
        nc.sync.dma_start(maskb[:], maskb_d[:])
        nc.vector.memset(ones1[:], 1.0)
        nc.vector.memset(ones128[:], 1.0)
        nc.vector.memset(epst[:], EPS)

        def layer_norm(y_t, out_t, out_off, gcol, bcol, vec_sb):
            """out_t[:, :, out_off + ...] = LN(y_t) * g + b  (over partition dim)."""
            for u in range(CHUNK // 512):
                usl = slice(u * 512, (u + 1) * 512)
                ysq = small.tile([128, HT, 512], DT16, tag="lnscratch")
                nc.vector.tensor_mul(ysq[:], y_t[:, :, usl], y_t[:, :, usl])
                st = pstat.tile([1, 2, 512], F32)
                for k in range(HT):
                    nc.tensor.matmul(st[:, 0, :], ones128[:], y_t[:, k, usl],
                                     start=(k == 0), stop=(k == HT - 1))
                for k in range(HT):
                    nc.tensor.matmul(st[:, 1, :], ones128[:], ysq[:, k, :],
                                     start=(k == 0), stop=(k == HT - 1))
                # epilogue on [1,512]
                mu = small.tile([1, 512], F32, tag="mu")
                t2 = small.tile([1, 512], F32, tag="t2")
                nc.vector.tensor_scalar(mu[:], st[:, 0, :], 1.0 / H, None, ALU.mult)
                nc.vector.tensor_scalar(t2[:], st[:, 1, :], 1.0 / H, None, ALU.mult)
                mm = small.tile([1, 512], F32, tag="rinv")
                nc.vector.tensor_mul(mm[:], mu[:], mu[:])
                nc.vector.tensor_sub(t2[:], t2[:], mm[:])
                nc.scalar.activation(t2[:], t2[:], AF.Sqrt, bias=epst[:])
                nc.vector.reciprocal(t2[:], t2[:])
                mub16 = small.tile([1, 512], DT16, tag="mub16")
                rsb16 = small.tile([1, 512], DT16, tag="rsb16")
                nc.vector.tensor_copy(mub16[:], mu[:])
                nc.vector.tensor_copy(rsb16[:], t2[:])
                mu_b = bcastp.tile([128, 512], DT16, tag="mu_b")
                rs_b = bcastp.tile([128, 512], DT16, tag="rs_b")
                nc.gpsimd.partition_broadcast(mu_b[:], mub16[:], channels=128)
                nc.gpsimd.partition_broadcast(rs_b[:], rsb16[:], channels=128)
                t1 = small.tile([128, HT, 512], DT16, tag="lnscratch")
                nc.vector.tensor_sub(
                    t1[:], y_t[:, :, usl],
                    mu_b[:, None, :].to_broadcast((128, HT, 512)))
                nc.vector.tensor_mul(
                    t1[:], t1[:], rs_b[:, None, :].to_broadcast((128, HT, 512)))
                for k in range(HT):
                    nc.scalar.activation(
                        out_t[:, k, out_off + u * 512: out_off + (u + 1) * 512],
                        t1[:, k, :], AF.Identity,
                        bias=vec_sb[:, bcol + k: bcol + k + 1],
                        scale=vec_sb[:, gcol + k: gcol + k + 1])

        with tc.For_i(0, n_layers) as l:
            vec_sb = vecp.tile([128, NVEC], F32)
            nc.sync.dma_start(vec_sb[:], vec_d[ds(l * 128, 128), :])
            bvr = vecp.tile([1, H], DT16, tag="bvr")
            nc.sync.dma_start(bvr[:], bvrow_d[ds(l, 1), :])

            for c in range(NCHUNK):
                t0 = c * CHUNK  # chunk column offset into x

                # ---------- Q and K (transposed) ----------
                qt = qkpool.tile([128, HT, CHUNK], DT16, tag="qt")
                kt = qkpool.tile([128, HT, CHUNK], DT16, tag="kt")
                for wi, (dst, bcol) in enumerate(((qt, C_BQ), (kt, C_BK))):
                    wtile = wmat.tile([128, HT, H], DT16, tag="wmat")
                    nc.sync.dma_start(
                        wtile[:], wqkvo_d[ds((l * 4 + wi) * H, H), :]
                        .rearrange("(c p) m -> p c m", p=128))
                    for n in range(CHUNK // 512):
                        for m in range(HT):
                            ps = pp.tile([128, 512], F32, tag="ps")
                            for k in range(HT):
                                nc.tensor.matmul(
                                    ps[:], wtile[:, k, m * 128:(m + 1) * 128],
                                    x[:, k, t0 + n * 512: t0 + (n + 1) * 512],
                                    start=(k == 0), stop=(k == HT - 1))
                            nc.scalar.activation(
                                dst[:, m, n * 512:(n + 1) * 512], ps[:], AF.Identity,
                                bias=vec_sb[:, bcol + m: bcol + m + 1])

                # ---------- V (natural layout, with ones column per head) ----------
                wv = wmat.tile([128, HT, H], DT16, tag="wmat")
                nc.sync.dma_start(
                    wv[:], wqkvo_d[ds((l * 4 + 2) * H, H), :]
                    .rearrange("(c p) m -> p c m", p=128))
                ve = vep.tile([128, CHUNK // 128, NH * 65], DT16)
                nc.vector.memset(ve[:, :, 64::65], 1.0)
                for v in range(CHUNK // 128):
                    for d in range(2):
                        ps = pp.tile([128, 384], F32, tag="ps")
                        for k in range(HT):
                            nc.tensor.matmul(
                                ps[:], x[:, k, t0 + v * 128: t0 + (v + 1) * 128],
                                wv[:, k, d * 384:(d + 1) * 384],
                                start=(k == 0), stop=False)
                        nc.tensor.matmul(
                            ps[:], ones1[:], bvr[:, d * 384:(d + 1) * 384],
                            start=False, stop=True)
                        nc.vector.tensor_copy(
                            ve[:, v, d * 390: d * 390 + 390]
                            .rearrange("p (h e) -> p h e", e=65)[:, :, 0:64],
                            ps[:].rearrange("p (h e) -> p h e", e=64))

                # ---------- attention ----------
                wo = wmat.tile([128, HT, H], DT16, tag="wmat")
                nc.sync.dma_start(
                    wo[:], wqkvo_d[ds((l * 4 + 3) * H, H), :]
                    .rearrange("(c p) m -> p c m", p=128))
                ctxt = ctxp.tile([128, HT, CHUNK], DT16)
                for s in range(2):
                    for h in range(NH):
                        pr, hc = (h % 2) * 64, h // 2
                        et = epool.tile([128, 4, 512], DT16)
                        for kk in range(4):
                            sc = pp.tile([128, 512], F32, tag="ps")
                            nc.tensor.matmul(
                                sc[:],
                                kt[pr:pr + 64, hc,
                                   s * 512 + kk * 128: s * 512 + (kk + 1) * 128],
                                qt[pr:pr + 64, hc, s * 512:(s + 1) * 512],
                                start=True, stop=True)
                            nc.scalar.activation(
                                et[:, kk, :], sc[:], AF.Exp,
                                bias=maskb[:, (c * 2 + s) * 4 + kk:
                                           (c * 2 + s) * 4 + kk + 1],
                                scale=0.125)
                        cx = pp.tile([65, 512], F32, tag="ps")
                        for kk in range(4):
                            nc.tensor.matmul(
                                cx[:], ve[:, s * 4 + kk, h * 65:(h + 1) * 65],
                                et[:, kk, :], start=(kk == 0), stop=(kk == 3))
                        rinv = small.tile([1, 512], F32, tag="rinv")
                        nc.vector.reciprocal(rinv[:], cx[64:65, :])
                        rb = bcastp.tile([64, 512], F32, tag="rb")
                        nc.gpsimd.partition_broadcast(rb[:], rinv[:], channels=64)
                        nc.vector.tensor_mul(
                            ctxt[pr:pr + 64, hc, s * 512:(s + 1) * 512],
                            cx[0:64, :], rb[:])

                # ---------- Wo + residual -> y, LN1 -> x1 ----------
                y = residp.tile([128, HT, CHUNK], DT16, tag="y")
                for n in range(CHUNK // 512):
                    for m in range(HT):
                        ps = pp.tile([128, 512], F32, tag="ps")
                        for k in range(HT):
                            nc.tensor.matmul(
                                ps[:], wo[:, k, m * 128:(m + 1) * 128],
                                ctxt[:, k, n * 512:(n + 1) * 512],
                                start=(k == 0), stop=(k == HT - 1))
                        nc.vector.scalar_tensor_tensor(
                            y[:, m, n * 512:(n + 1) * 512], ps[:],
                            vec_sb[:, C_BO + m: C_BO + m + 1],
                            x[:, m, t0 + n * 512: t0 + (n + 1) * 512],
                            op0=ALU.add, op1=ALU.add)
                x1 = x1p.tile([128, HT, CHUNK], DT16)
                layer_norm(y, x1, 0, C_L1G, C_L1B, vec_sb)

                # ---------- FFN ----------
                z = residp.tile([128, HT, CHUNK], DT16, tag="y")
                for u in range(CHUNK // 512):
                    usl = slice(u * 512, (u + 1) * 512)
                    ht = htp.tile([128, FT, 512], DT16)
                    w1c = None
                    for ft in range(FT):
                        if ft % 4 == 0:
                            w1c = w1p.tile([128, HT, 512], DT16, tag="w1c")
                            nc.sync.dma_start(
                                w1c[:],
                                w1_d[ds(l * H, H),
                                     (ft // 4) * 512:(ft // 4 + 1) * 512]
                                .rearrange("(c p) f -> p c f", p=128))
                        ps = pp.tile([128, 512], F32, tag="ps")
                        for k in range(HT):
                            nc.tensor.matmul(
                                ps[:], w1c[:, k, (ft % 4) * 128:(ft % 4 + 1) * 128],
                                x1[:, k, usl],
                                start=(k == 0), stop=(k == HT - 1))
                        nc.scalar.activation(
                            ht[:, ft, :], ps[:], AF.Gelu,
                            bias=vec_sb[:, C_B1 + ft: C_B1 + ft + 1])
                    # W2: two passes of 3 output h-tiles, PSUM held across f
                    for half in range(2):
                        zp = [pp.tile([128, 512], F32, tag="ps", name=f"zp{u}_{half}_{i}")
                              for i in range(3)]
                        w2c = None
                        for ft in range(FT):
                            if ft % 4 == 0:
                                w2c = w2p.tile([128, 4, H], DT16, tag="w2c")
                                nc.sync.dma_start(
                                    w2c[:],
                                    w2_d[ds(l * DFF + (ft // 4) * 512, 512), :]
                                    .rearrange("(c p) h -> p c h", p=128))
                            for mi in range(3):
                                m = half * 3 + mi
                                nc.tensor.matmul(
                                    zp[mi][:], w2c[:, ft % 4, m * 128:(m + 1) * 128],
                                    ht[:, ft, :],
                                    start=(ft == 0), stop=(ft == FT - 1))
                        for mi in range(3):
                            m = half * 3 + mi
                            nc.vector.scalar_tensor_tensor(
                                z[:, m, usl], zp[mi][:],
                                vec_sb[:, C_B2 + m: C_B2 + m + 1],
                                x1[:, m, usl], op0=ALU.add, op1=ALU.add)
                layer_norm(z, x, t0, C_L2G, C_L2B, vec_sb)

        # ---------------- output: CLS columns ----------------
        outsb = persist.tile([128, HT, B_LOC], F32)
        for m in range(HT):
            nc.vector.tensor_copy(outsb[:, m, :], x[:, m, 0::S])
        nc.sync.dma_start(out_d.rearrange("c p b -> p c b"), outsb[:])

    nc.compile()
    return nc


# ---------------------------------------------------------------------------
# host side
# ---------------------------------------------------------------------------

_CACHE = {}


def _to16(a):
    return np.asarray(a, dtype=np.float32).astype(np.float16)


def prepare_inputs(input_ids, attention_mask, token_type_ids, params):
    """Host preprocessing -> per-core in_maps."""
    p = {k: np.asarray(v, np.float32) for k, v in params.items()
         if k not in ("head_W", "head_b")}
    ids = np.asarray(input_ids).astype(np.int64)
    tt = np.asarray(token_type_ids).astype(np.int64)
    am = np.asarray(attention_mask).astype(np.float32)

    x0 = p["E_word"][ids] + p["E_pos"][None, :S] + p["E_type"][tt]  # [B,S,H] f32
    mu = x0.mean(-1, keepdims=True)
    var = ((x0 - mu) ** 2).mean(-1, keepdims=True)
    x0 = (x0 - mu) / np.sqrt(var + EPS) * p["emb_g"] + p["emb_b"]
    mask_bias = (1.0 - am) * -10000.0  # [B,S]

    # weights, packed once
    wqkvo = np.stack([p["Wq"], p["Wk"], p["Wv"], p["Wo"]], axis=1)  # [L,4,H,H]
    wqkvo = _to16(wqkvo.reshape(L * 4 * H, H))
    w1 = _to16(p["W1"].reshape(L * H, DFF))
    w2 = _to16(p["W2"].reshape(L * DFF, H))
    vec = np.zeros((L, 128, NVEC), np.float32)

    def put(col, arr_LD):  # arr [L, D] -> cols col..col+D/128
        d = arr_LD.shape[1]
        vec[:, :, col:col + d // 128] = arr_LD.reshape(L, d // 128, 128).transpose(0, 2, 1)

    put(C_BQ, p["bq"]); put(C_BK, p["bk"]); put(C_BV, p["bv"]); put(C_BO, p["bo"])
    put(C_B1, p["b1"]); put(C_B2, p["b2"])
    put(C_L1G, p["ln1_g"]); put(C_L1B, p["ln1_b"])
    put(C_L2G, p["ln2_g"]); put(C_L2B, p["ln2_b"])
    vec = vec.reshape(L * 128, NVEC)
    bvrow = _to16(p["bv"])

    in_maps = []
    for c in range(NCORES):
        sl = slice(c * B_LOC, (c + 1) * B_LOC)
        x0c = x0[sl].reshape(T, H).T  # [H, T]
        mb = mask_bias[sl].reshape(B_LOC, 4, 128).transpose(2, 0, 1).reshape(128, B_LOC * 4)
        in_maps.append({
            "x0": _to16(np.ascontiguousarray(x0c)),
            "wqkvo": wqkvo, "w1": w1, "w2": w2, "vec": vec, "bvrow": bvrow,
            "maskb": np.ascontiguousarray(mb),
        })
    return in_maps


def finish(results, annotator_idx, params):
    """Gather per-core CLS vectors, apply annotator head on host."""
    ann = np.asarray(annotator_idx).astype(np.int64)
    hw = np.asarray(params["head_W"], np.float32)
    hb = np.asarray(params["head_b"], np.float32)
    cls = np.concatenate(
        [r["out"].transpose(2, 0, 1).reshape(B_LOC, H) for r in results], axis=0)
    return np.einsum("bh,bhc->bc", cls, hw[ann]) + hb[ann]


def kernel(input_ids, attention_mask, token_type_ids, annotator_idx, params):
    if "nc" not in _CACHE:
        _CACHE["nc"] = build_nc()
    nc = _CACHE["nc"]
    in_maps = prepare_inputs(input_ids, attention_mask, token_type_ids, params)
    res = run_bass_kernel_spmd(nc, in_maps, core_ids=list(range(NCORES)))
    return finish(res.results, annotator_idx, params).astype(np.float32)


# ---------------------------------------------------------------------------
# timing harness (mirrors bass2jax.run_bass_via_pjrt but keeps a persistent
# jitted callable + device-resident inputs so repeat executions time the NEFF)
# ---------------------------------------------------------------------------

def make_timed_runner(nc, in_maps):
    import jax
    from jax.sharding import Mesh, PartitionSpec, NamedSharding
    from jax.experimental.shard_map import shard_map
    from concourse import bass2jax

    bass2jax.install_neuronx_cc_hook()
    n_cores = len(in_maps)

    pname = nc.partition_id_tensor.name if nc.partition_id_tensor else None
    in_names, out_names, out_avals, zero_outs = [], [], [], []
    for alloc in nc.m.functions[0].allocations:
        if not isinstance(alloc, mybir.MemoryLocationSet):
            continue
        name = alloc.memorylocations[0].name
        if alloc.kind == "ExternalInput":
            if name != pname:
                in_names.append(name)
        elif alloc.kind == "ExternalOutput":
            out_names.append(name)
            shape = tuple(alloc.tensor_shape)
            dtype = mybir.dt.np(alloc.dtype)
            out_avals.append(jax.core.ShapedArray(shape, dtype))
            zero_outs.append(np.zeros(shape, dtype))
    n_params = len(in_names)
    all_names = in_names + out_names + ([pname] if pname else [])

    def _body(*args):
        args = list(args)
        if pname:
            args.append(bass2jax.partition_id_tensor())
        outs = bass2jax._bass_exec_p.bind(
            *args, out_avals=tuple(out_avals), in_names=tuple(all_names),
            out_names=tuple(out_names), lowering_input_output_aliases=(),
            sim_require_finite=True, sim_require_nnan=True, nc=nc)
        return tuple(outs)

    devices = jax.devices()[:n_cores]
    mesh = Mesh(np.asarray(devices), ("core",))
    sharded = jax.jit(shard_map(
        _body, mesh=mesh,
        in_specs=(PartitionSpec("core"),) * (n_params + len(out_names)),
        out_specs=(PartitionSpec("core"),) * len(out_names), check_rep=False))

    sh = NamedSharding(mesh, PartitionSpec("core"))
    dev_args = [
        jax.device_put(
            np.concatenate([np.asarray(in_maps[c][k]) for c in range(n_cores)], 0), sh)
        for k in in_names
    ] + [
        jax.device_put(np.zeros((n_cores * z.shape[0], *z.shape[1:]), z.dtype), sh)
        for z in zero_outs
    ]

    def run():
        return [o.block_until_ready() for o in sharded(*dev_args)]

    return run, out_names, out_avals


def timed_run(inputs, reps=10):
    import time
    if "nc" not in _CACHE:
        _CACHE["nc"] = build_nc()
    nc = _CACHE["nc"]
    in_maps = prepare_inputs(
        inputs["input_ids"], inputs["attention_mask"],
        inputs["token_type_ids"], inputs["params"])
    run, _, _ = make_timed_runner(nc, in_maps)
    run()  # compile + warm
    run()
    ts = []
    for _ in range(reps):
        t0 = time.perf_counter()
        run()
        ts.append(time.perf_counter() - t0)
    ts = np.array(ts) * 1e9
    print(f"per-call wall ns: min {ts.min():.0f}  median {np.median(ts):.0f}  "
          f"mean {ts.mean():.0f}")
    return float(np.median(ts))
